# revision 28
# baseline (speedup 1.0000x reference)
"""Trainium2 Bass kernel v2 for a pre-norm transformer block.

Data-parallel B=8 over 8 cores. Per-core, activations transposed [feat, tok].

Changes vs v1 baseline:
- fp8e4 DoubleRow matmuls (K=256/instr) for qkv q/k, v, attnV, proj, fc1, fc2.
  Weights scaled x256 (x64 for v) on host to clear e4m3 subnormals; scales
  unwound at PSUM eviction (or cancel in softmax normalization).
- LN stats matmuls read fp32 directly as float32r (full rate, no bf16 copies).
- rstd = exp(-0.5*ln(var+eps)) on ACT: the natural_log_exp table set serves
  the whole kernel until the single switch to gelu. LN beta folded into the
  effective bias of downstream matmuls on host (h = (x-mu)*rstd*g on device).
- exp over paired key-tiles [128,2,1024] (halves ACT instruction overhead),
  writes fp8 in the DoubleRow pair layout consumed by attnV.
- Per-head streaming normalize: DVE reciprocal of the denominator row read
  straight from PSUM, K=1 PE broadcast, one TT mult -> oT fp8.
"""

import numpy as np
import ml_dtypes

EMBED = 1024
HEADS = 16
HIDDEN = 4096
N_TOK = 1024
B = 8
N_CORES = 8
EPS = 1e-5
P = 128
CSUB = EMBED // P          # 8
HSUB = HIDDEN // P         # 32
QT = 2
QW = 512

F8 = ml_dtypes.float8_e4m3
WS = 256.0                 # weight scale for qkv(qk)/proj/fc1/fc2
VS = 64.0                  # weight scale for v path (ones col = 64 cancels)

_CACHE = {}
GELU = True      # debug: False -> Identity
TAPS = False     # debug: extra dram taps


# ---------------------------------------------------------------------------
# host-side packing
# ---------------------------------------------------------------------------

def _pack_dr(w, scale):
    """[K, M] fp32 -> [M//128, 128, K//256, 2, 128] fp8 DoubleRow chunks.

    chunk[mt][p, kp, i, mi] = w[(2*kp+i)*128 + p, mt*128 + mi] * scale
    """
    K, M = w.shape
    a = w.reshape(K // 256, 2, P, M // P, P).transpose(3, 2, 0, 1, 4)
    return np.ascontiguousarray((a * scale).astype(F8))


def _pack_rhs8(w, scale):
    """[K, M] fp32 -> [128, K//128, M] fp8 (moving layout)."""
    K, M = w.shape
    a = w.reshape(K // P, P, M).transpose(1, 0, 2)
    return np.ascontiguousarray((a * scale).astype(F8))


def _pack_percol(v):
    F = v.shape[0]
    return np.ascontiguousarray(v.reshape(F // P, P).T.astype(np.float32))


def _pack_xT(xb):
    xT = xb.T
    a = xT.reshape(CSUB, P, N_TOK).transpose(1, 0, 2)
    return np.ascontiguousarray(a.astype(np.float32))


def _unpack_yT(yT):
    full = yT.transpose(1, 0, 2).reshape(EMBED, N_TOK)
    return np.ascontiguousarray(full.T)


# ---------------------------------------------------------------------------
# kernel build
# ---------------------------------------------------------------------------

def _build():
    import concourse.bacc as bacc
    import concourse.mybir as mybir
    import concourse.tile as tile
    from contextlib import ExitStack

    dt = mybir.dt
    AF = mybir.ActivationFunctionType
    OP = mybir.AluOpType
    DR = mybir.MatmulPerfMode.DoubleRow

    nc = bacc.Bacc("TRN2", target_bir_lowering=False, debug=False)

    f32, bf16, f8 = dt.float32, dt.bfloat16, dt.float8e4

    def dram(name, shape, d=f32, out=False):
        return nc.dram_tensor(name, list(shape), d,
                              kind="ExternalOutput" if out else "ExternalInput").ap()

    xT_d = dram("xT", [P, CSUB, N_TOK])
    wqk_d = dram("wqk", [16, P, 4, 2, P], f8)
    bqk_d = dram("bqk", [P, 16])
    wv_d = dram("wv", [P, CSUB, EMBED], f8)
    bv_d = dram("bv", [1, EMBED], bf16)
    wpr_d = dram("wpr", [CSUB, P, 4, 2, P], f8)
    bpr_d = dram("bpr", [P, CSUB])
    wf1_d = dram("wf1", [HSUB, P, 4, 2, P], f8)
    bf1_d = dram("bf1", [P, HSUB])
    wf2_d = dram("wf2", [CSUB, P, 16, 2, P], f8)
    bf2_d = dram("bf2", [P, CSUB])
    g1_d = dram("g1", [P, CSUB])
    g2_d = dram("g2", [P, CSUB])
    yT_d = dram("yT", [P, CSUB, N_TOK], out=True)
    if TAPS:
        vps_t = dram("vps_t", [P, N_TOK], out=True)
        vbb_t = dram("vbb_t", [P, EMBED], out=True)
        wv_t = dram("wv_t", [P, CSUB, EMBED], f8, out=True)
        hT_t = dram("hT_t", [P, CSUB, N_TOK], f8, out=True)
        qkp_t = dram("qkp_t", [P, 8, 2, N_TOK], bf16, out=True)
        v65_t = dram("v65_t", [P, CSUB, HEADS, 65], f8, out=True)
        oT_t = dram("oT_t", [P, CSUB, N_TOK], f8, out=True)
        x1_t = dram("x1_t", [P, CSUB, N_TOK], out=True)
        ln2_t = dram("ln2_t", [P, CSUB, N_TOK], f8, out=True)

    with tile.TileContext(nc) as tc, ExitStack() as ctx:
        const = ctx.enter_context(tc.tile_pool(name="const", bufs=1))
        persist = ctx.enter_context(tc.tile_pool(name="persist", bufs=1))
        smalls = ctx.enter_context(tc.tile_pool(name="smalls", bufs=1))
        tmpf = ctx.enter_context(tc.tile_pool(name="tmpf", bufs=2))
        wpool = ctx.enter_context(tc.tile_pool(name="wpool", bufs=2))

        # ---- constants ---------------------------------------------------
        ones_mm = const.tile([P, P], bf16)      # 1/1024 for LN mean
        nc.vector.memset(ones_mm[:], 1.0 / EMBED)
        ones_bc = const.tile([65, P], bf16)     # K=1 broadcast rows
        nc.vector.memset(ones_bc[:], 1.0)

        bqk_sb = const.tile([P, 16], f32)
        nc.sync.dma_start(bqk_sb[:], bqk_d[:])
        bv_row = const.tile([1, EMBED], bf16)
        nc.sync.dma_start(bv_row[:], bv_d[:])
        bpr_sb = const.tile([P, CSUB], f32)
        nc.sync.dma_start(bpr_sb[:], bpr_d[:])
        bf1_sb = const.tile([P, HSUB], f32)
        nc.sync.dma_start(bf1_sb[:], bf1_d[:])
        bf2_sb = const.tile([P, CSUB], f32)
        nc.sync.dma_start(bf2_sb[:], bf2_d[:])
        g1_sb = const.tile([P, CSUB], f32)
        nc.sync.dma_start(g1_sb[:], g1_d[:])
        g2_sb = const.tile([P, CSUB], f32)
        nc.sync.dma_start(g2_sb[:], g2_d[:])

        xT = persist.tile([P, CSUB, N_TOK], f32)
        for c in range(CSUB):
            nc.sync.dma_start(xT[:, c, :], xT_d[:, c, :])

        # ---- layernorm: out_fp8 = (x - mu) * rstd * g  (beta folded) -----
        def emit_layernorm(ps, x_sb, g_col, out_sb, tag, step_cb=None):
            mu_ps = ps.tile([P, N_TOK], f32, tag="ps")
            sq_ps = ps.tile([P, N_TOK], f32, tag="ps")
            for c in range(CSUB):
                x_b = tmpf.tile([P, N_TOK], bf16, tag="xb", bufs=2)
                nc.vector.tensor_copy(x_b[:], x_sb[:, c, :])
                sq_t = tmpf.tile([P, N_TOK], bf16, tag="sq", bufs=2)
                nc.scalar.activation(sq_t[:], x_sb[:, c, :], AF.Square)
                for q in range(QT):
                    sl = slice(q * QW, (q + 1) * QW)
                    nc.tensor.matmul(mu_ps[:, sl], ones_mm[:], x_b[:, sl],
                                     start=(c == 0), stop=(c == CSUB - 1))
                    nc.tensor.matmul(sq_ps[:, sl], ones_mm[:], sq_t[:, sl],
                                     start=(c == 0), stop=(c == CSUB - 1))
            mu2 = smalls.tile([P, N_TOK], f32, tag="mu2")
            var_t = smalls.tile([P, N_TOK], f32, tag="var")
            lnv = smalls.tile([P, N_TOK], f32, tag="lnv")
            rstd = smalls.tile([P, N_TOK], f32, tag="rstd")
            for q in range(QT):
                sl = slice(q * QW, (q + 1) * QW)
                nc.scalar.activation(mu2[:, sl], mu_ps[:, sl], AF.Square)
                nc.vector.scalar_tensor_tensor(var_t[:, sl], sq_ps[:, sl],
                                               EPS, mu2[:, sl],
                                               OP.add, OP.subtract)
                nc.scalar.activation(lnv[:, sl], var_t[:, sl], AF.Ln)
                nc.scalar.activation(rstd[:, sl], lnv[:, sl], AF.Exp,
                                     scale=-0.5)
            mu1_sb = smalls.tile([P, N_TOK], f32, tag="mu2", name="mu1sb")
            nc.vector.tensor_copy(mu1_sb[:], mu_ps[:])
            for cp in range(4):
                for c in (2 * cp, 2 * cp + 1):
                    t = tmpf.tile([P, N_TOK], f32, tag="lnt", bufs=2)
                    for qh in range(QT):
                        sl = slice(qh * QW, (qh + 1) * QW)
                        nc.vector.tensor_tensor(t[:, sl], x_sb[:, c, sl],
                                                mu1_sb[:, sl], OP.subtract)
                        nc.vector.scalar_tensor_tensor(
                            out_sb[:, c, sl], t[:, sl], g_col[:, c:c + 1],
                            rstd[:, sl], OP.mult, OP.mult)
                if step_cb is not None:
                    step_cb(cp)

        # =================================================================
        # phase A: LN1 -> hT fp8, then all of q,k,v (own PSUM pool)
        # =================================================================
        attn_sb = ctx.enter_context(tc.tile_pool(name="attn_sb", bufs=1))
        v65 = attn_sb.tile([P, CSUB, HEADS, 65], f8)
        oT = attn_sb.tile([P, CSUB, N_TOK], f8)
        qkp = attn_sb.tile([P, 8, 2, N_TOK], bf16)   # [., pair, q/k, tok]
        pro_e = {}

        with tc.tile_pool(name="psA", bufs=4, space="PSUM") as psA, \
             tc.tile_pool(name="hTp", bufs=1) as hTp, \
             tc.tile_pool(name="wv_sb", bufs=1) as wvp:
            hT = hTp.tile([P, CSUB, N_TOK], f8)
            PRE_M = (0, 8, 1, 9)
            pre_w, pre_ps = [], []
            for i, m in enumerate(PRE_M):
                w = wpool.tile([P, 4, 2, P], f8, tag="wf1k", bufs=6,
                               name=f"wq{i}")
                nc.scalar.dma_start(w[:], wqk_d[m])
                pre_w.append(w)

            def qk_cb(cp):
                if cp == 0:
                    for i in range(4):
                        pre_ps.append(psA.tile([P, N_TOK], f32, tag="ps",
                                               name=f"pq{i}"))
                for i in range(4):
                    for q in range(QT):
                        sl = slice(q * QW, (q + 1) * QW)
                        nc.tensor.matmul(pre_ps[i][:, sl],
                                         pre_w[i][:, cp, :, :],
                                         hT[:, 2 * cp:2 * cp + 2, sl],
                                         start=(cp == 0), stop=(cp == 3),
                                         perf_mode=DR)

            emit_layernorm(psA, xT, g1_sb, hT, "1", step_cb=qk_cb)
            for i, m in enumerate(PRE_M):
                nc.scalar.activation(qkp[:, m % 8, m // 8, :], pre_ps[i][:],
                                     AF.Identity, bias=bqk_sb[:, m:m + 1])
            # residual pre-bias: xT += bpr, after LN1 consumed xT
            for c in range(CSUB):
                nc.vector.tensor_scalar(xT[:, c, :], xT[:, c, :],
                                        bpr_sb[:, c:c + 1], None, OP.add)

            # ---- q,k for all 8 head-pairs -------------------------------
            for hp in range(2, 8):
                for i, m in enumerate((hp, 8 + hp)):
                    wch = wpool.tile([P, 4, 2, P], f8, tag="w1k")
                    nc.sync.dma_start(wch[:], wqk_d[m])
                    qk_ps = psA.tile([P, N_TOK], f32, tag="ps")
                    for kp in range(4):
                        for q in range(QT):
                            sl = slice(q * QW, (q + 1) * QW)
                            nc.tensor.matmul(qk_ps[:, sl], wch[:, kp, :, :],
                                             hT[:, 2 * kp:2 * kp + 2, sl],
                                             start=(kp == 0), stop=(kp == 3),
                                             perf_mode=DR)
                    nc.scalar.activation(qkp[:, hp, i, :], qk_ps[:],
                                         AF.Identity,
                                         bias=bqk_sb[:, m:m + 1])

            # ---- v = hT' wv + bias, ones col = 64 -----------------------
            wv_sb = wvp.tile([P, CSUB, EMBED], f8)
            nc.sync.dma_start(wv_sb[:], wv_d[:])
            vb_ps = psA.tile([P, N_TOK], f32, tag="ps")
            for q in range(QT):
                sl = slice(q * QW, (q + 1) * QW)
                nc.tensor.matmul(vb_ps[:, sl], ones_bc[0:1, :],
                                 bv_row[:, sl])
            vb_b = wvp.tile([P, EMBED], f32)
            nc.vector.tensor_copy(vb_b[:], vb_ps[:])
            if TAPS:
                nc.sync.dma_start(vbb_t[:], vb_b[:])
                nc.sync.dma_start(wv_t[:], wv_sb[:])

            nc.vector.memset(v65[:, :, :, 64:65], 64.0)
            pro_specs = [(0, 0), (0, 1), (0, 2), (0, 3), (1, 0)]
            for mt in range(CSUB):
                v_ps = psA.tile([P, N_TOK], f32, tag="ps")
                for kp in range(4):
                    for q in range(QT):
                        sl = slice(q * QW, (q + 1) * QW)
                        nc.tensor.matmul(
                            v_ps[:, sl],
                            hT[:, 2 * kp:2 * kp + 2, mt * P:(mt + 1) * P],
                            wv_sb[:, 2 * kp:2 * kp + 2, sl],
                            start=(kp == 0), stop=(kp == 3), perf_mode=DR)
                nc.vector.tensor_tensor(
                    v65[:, mt, :, 0:64],
                    v_ps[:].rearrange("p (h d) -> p h d", d=64),
                    vb_b[:].rearrange("p (h d) -> p h d", d=64),
                    OP.add)
                if mt >= 3 and pro_specs:
                    ph, ptp = pro_specs.pop(0)
                    pbs = slice((ph % 2) * 64, (ph % 2) * 64 + 64)
                    e_t = tmpf.tile([P, 2, N_TOK], f8, tag="exp", bufs=6,
                                    name="pet")
                    for j in range(2):
                        k = 2 * ptp + j
                        s_ps = psA.tile([P, N_TOK], f32, tag="ps")
                        for q in range(QT):
                            sl = slice(q * QW, (q + 1) * QW)
                            nc.tensor.matmul(
                                s_ps[:, sl],
                                qkp[pbs, 0, 1, k * P:(k + 1) * P],
                                qkp[pbs, 0, 0, sl])
                        nc.scalar.activation(e_t[:, j, :], s_ps[:], AF.Exp,
                                             scale=0.125 / (WS * WS))
                    pro_e[(ph, ptp)] = e_t

        if TAPS:
            nc.sync.dma_start(hT_t[:], hT[:])
            nc.sync.dma_start(qkp_t[:], qkp[:])
            nc.sync.dma_start(v65_t[:], v65[:])
        # =================================================================
        # phase B: attention scores/expr/attnV/normalize
        # =================================================================
        oU = attn_sb.tile([64, 8, N_TOK], bf16)
        drow = attn_sb.tile([65, N_TOK], bf16)     # row 64: denom (bf16)
        bc_sb = attn_sb.tile([64, N_TOK], f32)     # 1/denom broadcast
        n64 = attn_sb.tile([64, N_TOK], f8)        # odd-head staging

        with tc.tile_pool(name="sp", bufs=3, space="PSUM") as sp, \
             tc.tile_pool(name="pop", bufs=1, space="PSUM") as pop:
            for h in range(HEADS):
                hp = h // 2
                bs = slice((h % 2) * 64, (h % 2) * 64 + 64)
                o_ps = pop.tile([65, N_TOK], f32, tag="po", name="ops")
                for tp in range(4):
                    if (h, tp) in pro_e:
                        e_t = pro_e[(h, tp)]
                    else:
                        e_t = tmpf.tile([P, 2, N_TOK], f8, tag="exp", bufs=6)
                        for j in range(2):
                            k = 2 * tp + j
                            s_ps = sp.tile([P, N_TOK], f32, tag="sp")
                            for q in range(QT):
                                sl = slice(q * QW, (q + 1) * QW)
                                nc.tensor.matmul(
                                    s_ps[:, sl],
                                    qkp[bs, hp, 1, k * P:(k + 1) * P],
                                    qkp[bs, hp, 0, sl])
                            nc.scalar.activation(e_t[:, j, :], s_ps[:],
                                                 AF.Exp,
                                                 scale=0.125 / (WS * WS))
                    for q in range(QT):
                        sl = slice(q * QW, (q + 1) * QW)
                        nc.tensor.matmul(o_ps[:, sl],
                                         v65[:, 2 * tp:2 * tp + 2, h, :],
                                         e_t[:, :, sl],
                                         start=(tp == 0), stop=(tp == 3),
                                         perf_mode=DR)
                # release o_ps fast: oU + denom-row copies; invert later
                nc.vector.tensor_copy(oU[:, h % 8, :], o_ps[0:64, :])
                nc.vector.tensor_copy(drow[64:65, :], o_ps[64:65, :])
                bc_ps = pop.tile([P, N_TOK], f32, tag="po", name="bcps")
                for q in range(QT):
                    sl = slice(q * QW, (q + 1) * QW)
                    nc.tensor.matmul(bc_ps[0:64, sl],
                                     ones_bc[64:65, 0:64],
                                     drow[64:65, sl])
                nc.vector.reciprocal_approx_fast(bc_sb[:], bc_ps[0:64, :])
                if h % 2 == 0:
                    nc.vector.tensor_tensor(oT[0:64, h // 2, :],
                                            oU[:, h % 8, :], bc_sb[:],
                                            OP.mult)
                else:
                    nc.vector.tensor_tensor(n64[:], oU[:, h % 8, :],
                                            bc_sb[:], OP.mult)
                    nc.sync.dma_start(oT[64:128, h // 2, :], n64[:])

        # =================================================================
        # phase C: proj + residual, LN2, MLP + residual -> yT
        # =================================================================
        with tc.tile_pool(name="mlp_sb", bufs=1) as mlp_sb, \
             tc.tile_pool(name="w4k", bufs=2) as w4k:
          with tc.tile_pool(name="psB", bufs=2, space="PSUM") as psB:
            ln2T = mlp_sb.tile([P, CSUB, N_TOK], f8)
            mu_ps = psB.tile([P, N_TOK], f32, tag="st", bufs=2)
            sq_ps = psB.tile([P, N_TOK], f32, tag="st", bufs=2)
            for m in range(CSUB):
                wch = wpool.tile([P, 4, 2, P], f8, tag="w1k")
                nc.sync.dma_start(wch[:], wpr_d[m])
                p_ps = psB.tile([P, N_TOK], f32, tag="ps")
                for kp in range(4):
                    for q in range(QT):
                        sl = slice(q * QW, (q + 1) * QW)
                        nc.tensor.matmul(p_ps[:, sl], wch[:, kp, :, :],
                                         oT[:, 2 * kp:2 * kp + 2, sl],
                                         start=(kp == 0), stop=(kp == 3),
                                         perf_mode=DR)
                nc.vector.scalar_tensor_tensor(xT[:, m, :], p_ps[:], 1.0 / WS,
                                               xT[:, m, :], OP.mult, OP.add)
                x_b = tmpf.tile([P, N_TOK], bf16, tag="xb", bufs=2)
                nc.vector.tensor_copy(x_b[:], xT[:, m, :])
                sq_t = tmpf.tile([P, N_TOK], bf16, tag="sq", bufs=2)
                nc.scalar.activation(sq_t[:], xT[:, m, :], AF.Square)
                for q in range(QT):
                    sl = slice(q * QW, (q + 1) * QW)
                    nc.tensor.matmul(mu_ps[:, sl], ones_mm[:], x_b[:, sl],
                                     start=(m == 0), stop=(m == CSUB - 1))
                    nc.tensor.matmul(sq_ps[:, sl], ones_mm[:], sq_t[:, sl],
                                     start=(m == 0), stop=(m == CSUB - 1))
            mu2 = smalls.tile([P, N_TOK], f32, tag="mu2")
            var_t = smalls.tile([P, N_TOK], f32, tag="var")
            lnv = smalls.tile([P, N_TOK], f32, tag="lnv")
            rstd = smalls.tile([P, N_TOK], f32, tag="rstd")
            for q in range(QT):
                sl = slice(q * QW, (q + 1) * QW)
                nc.scalar.activation(mu2[:, sl], mu_ps[:, sl], AF.Square)
                nc.vector.scalar_tensor_tensor(var_t[:, sl], sq_ps[:, sl],
                                               EPS, mu2[:, sl],
                                               OP.add, OP.subtract)
                nc.scalar.activation(lnv[:, sl], var_t[:, sl], AF.Ln)
                nc.scalar.activation(rstd[:, sl], lnv[:, sl], AF.Exp,
                                     scale=-0.5)
            mu_sb = smalls.tile([P, N_TOK], f32, tag="mu2")
            nc.vector.tensor_copy(mu_sb[:], mu_ps[:])

          with tc.tile_pool(name="psC", bufs=4, space="PSUM") as psC:
            geluT = mlp_sb.tile([P, HSUB, N_TOK], f8)
            # LN2 normalize interleaved with the first 4 fc1 m-tile chains
            wchs, fpss = [], []
            for m in range(4):
                w = wpool.tile([P, 4, 2, P], f8, tag="wf1k", bufs=6,
                               name=f"wf{m}")
                nc.sync.dma_start(w[:], wf1_d[m])
                wchs.append(w)
                fpss.append(psC.tile([P, N_TOK], f32, tag="ps",
                                     name=f"fp{m}"))
            for cp in range(4):
                for c in (2 * cp, 2 * cp + 1):
                    t = tmpf.tile([P, N_TOK], f32, tag="lnt", bufs=2)
                    for qh in range(QT):
                        sl = slice(qh * QW, (qh + 1) * QW)
                        nc.vector.tensor_tensor(t[:, sl], xT[:, c, sl],
                                                mu_sb[:, sl], OP.subtract)
                        nc.vector.scalar_tensor_tensor(
                            ln2T[:, c, sl], t[:, sl], g2_sb[:, c:c + 1],
                            rstd[:, sl], OP.mult, OP.mult)
                for m in range(4):
                    for q in range(QT):
                        sl = slice(q * QW, (q + 1) * QW)
                        nc.tensor.matmul(fpss[m][:, sl],
                                         wchs[m][:, cp, :, :],
                                         ln2T[:, 2 * cp:2 * cp + 2, sl],
                                         start=(cp == 0), stop=(cp == 3),
                                         perf_mode=DR)
            for c in range(CSUB):
                nc.vector.tensor_scalar(xT[:, c, :], xT[:, c, :],
                                        bf2_sb[:, c:c + 1], None, OP.add)
            for m in range(4):
                nc.scalar.activation(geluT[:, m, :], fpss[m][:],
                                     AF.Gelu if GELU else AF.Identity,
                                     bias=bf1_sb[:, m:m + 1], scale=1.0 / WS)
            for m in range(4, HSUB):
                wch = wpool.tile([P, 4, 2, P], f8, tag="wf1k", bufs=6)
                nc.sync.dma_start(wch[:], wf1_d[m])
                f_ps = psC.tile([P, N_TOK], f32, tag="ps")
                for kp in range(4):
                    for q in range(QT):
                        sl = slice(q * QW, (q + 1) * QW)
                        nc.tensor.matmul(f_ps[:, sl], wch[:, kp, :, :],
                                         ln2T[:, 2 * kp:2 * kp + 2, sl],
                                         start=(kp == 0), stop=(kp == 3),
                                         perf_mode=DR)
                nc.scalar.activation(geluT[:, m, :], f_ps[:],
                                     AF.Gelu if GELU else AF.Identity,
                                     bias=bf1_sb[:, m:m + 1], scale=1.0 / WS)

            for m2 in range(CSUB):
                w2ch = w4k.tile([P, 16, 2, P], f8, tag="w4k")
                nc.sync.dma_start(w2ch[:], wf2_d[m2])
                y_ps = psC.tile([P, N_TOK], f32, tag="ps")
                for kp in range(16):
                    for q in range(QT):
                        sl = slice(q * QW, (q + 1) * QW)
                        nc.tensor.matmul(y_ps[:, sl], w2ch[:, kp, :, :],
                                         geluT[:, 2 * kp:2 * kp + 2, sl],
                                         start=(kp == 0), stop=(kp == 15),
                                         perf_mode=DR)
                for q in range(QT):
                    sl = slice(q * QW, (q + 1) * QW)
                    nc.vector.scalar_tensor_tensor(
                        xT[:, m2, sl], y_ps[:, sl], 1.0 / WS,
                        xT[:, m2, sl], OP.mult, OP.add)
                    nc.sync.dma_start(yT_d[:, m2, sl], xT[:, m2, sl])

    nc.compile()
    return nc


def get_nc():
    if "nc" not in _CACHE:
        _CACHE["nc"] = _build()
    return _CACHE["nc"]


def make_in_maps(x, qkv_w, qkv_b, proj_w, proj_b, fc1_w, fc1_b, fc2_w, fc2_b,
                 ln1_g, ln1_b, ln2_g, ln2_b):
    x = np.asarray(x, np.float32)
    qkv_w = np.asarray(qkv_w, np.float32)
    qkv_b = np.asarray(qkv_b, np.float32)
    fc1_w = np.asarray(fc1_w, np.float32)
    ln1_b = np.asarray(ln1_b, np.float32)
    ln2_b = np.asarray(ln2_b, np.float32)
    # fold LN betas into downstream effective biases (h = (x-mu)*rstd*g dev)
    bqk_eff = (qkv_b[:2048] + ln1_b @ qkv_w[:, :2048]) * WS
    bv_eff = (qkv_b[2048:] + ln1_b @ qkv_w[:, 2048:]) * VS
    bf1_eff = np.asarray(fc1_b, np.float32) + ln2_b @ fc1_w
    shared = {
        "wqk": _pack_dr(qkv_w[:, :2048], WS),
        "bqk": _pack_percol(bqk_eff),
        "wv": _pack_rhs8(qkv_w[:, 2048:], VS),
        "bv": np.ascontiguousarray(bv_eff[None, :].astype(ml_dtypes.bfloat16)),
        "wpr": _pack_dr(np.asarray(proj_w, np.float32), WS),
        "bpr": _pack_percol(np.asarray(proj_b, np.float32)),
        "wf1": _pack_dr(fc1_w, WS),
        "bf1": _pack_percol(bf1_eff),
        "wf2": _pack_dr(np.asarray(fc2_w, np.float32), WS),
        "bf2": _pack_percol(np.asarray(fc2_b, np.float32)),
        "g1": _pack_percol(np.asarray(ln1_g, np.float32)),
        "g2": _pack_percol(np.asarray(ln2_g, np.float32)),
    }
    return [dict(shared, xT=_pack_xT(x[b])) for b in range(B)]


def kernel(**inputs):
    from concourse.bass_utils import run_bass_kernel_spmd

    nc = get_nc()
    in_maps = make_in_maps(**inputs)
    res = run_bass_kernel_spmd(nc, in_maps, core_ids=list(range(N_CORES)))
    out = np.stack([_unpack_yT(res.results[b]["yT"]) for b in range(B)])
    return out.astype(np.float32)
